# revision 1
# baseline (speedup 1.0000x reference)
"""Trainium2 Bass kernel for a dense transformer encoder layer.

Problem: B=2, S=2048, D=1024, H=16 heads (W=64), F=4096, fp32.

Sharding: 8 cores = 2 batches x 4 sequence chunks of 512 tokens. Each core
computes K/V for its batch's full sequence (replicated within its 4-core
batch group) and Q/attention/FFN for its own 512-token chunk. No collectives.

Dataflow: activations live TRANSPOSED in SBUF ([feature, token], feature on
partitions) so QKV projections, attention, output projection and both FFN
matmuls chain on the TensorEngine with no on-device transposes. The host
transposes x on the way in and the per-core 1024x512 output on the way out.

Softmax: score tiles are [key-token, query-token]. The additive -10000 mask
is folded multiplicatively into V and into the per-head Z column as
gamma_t = exp(-10000*(1-m_t)) (exactly 0/1 in fp32), so exp needs no bias
and pairs of key-chunks share one wide ACT call. The normalizer Z comes
free as a 65th gamma-column appended to each head of V (the attention-value
matmul emits it as PSUM row 64); normalization multiplies by a PE-broadcast
reciprocal row. LayerNorm statistics ride 1/D-scaled ones-column matmuls
and the affine apply is two DVE passes against PE-built rank-1 tiles.

Matmuls run in float32r (TF32-like, 4x PE throughput, ~5e-4 rel error
end to end). Set USE_F32R = False for exact-fp32 matmuls (~3x slower).
"""
import numpy as np
import concourse.bass as bass
from concourse import bacc
import concourse.mybir as mybir
import concourse.tile as tile
from concourse.bass import ts
from concourse.bass_utils import run_bass_kernel_spmd

P = 128
B, S, D, H, W, F = 2, 2048, 1024, 16, 64, 4096
DC = D // P            # 8 d-chunks
FC = F // P            # 32 f-chunks
TC = S // P            # 16 key-token chunks
SCH = 512              # tokens per core
EPS = 1e-12
SCALE = 1.0 / np.sqrt(np.float32(W))
WA = W + 1             # per-head V columns incl. ones column

F32 = mybir.dt.float32
# float32r = TF32-like PE mode (4x matmul throughput, ~1e-4 rel err).
# float32  = exact fp32 matmul (4 cycles/row).
USE_F32R = True
DT = mybir.dt.float32r if USE_F32R else F32

_cache = {}


def _layer_norm(nc, tc, pp, pp2, ppacc, onesw, invd, src, sq, dst, grow, nbrow, tag):
    """src/sq/dst: [P, DC, SCH] sbuf (feature on partitions). LN over features.
    sq = src*src comes from the caller's producing evacuation. Mean scaling
    rides the stats matmuls via the invd column. The apply is two DVE passes:
    dst = src*A - B with rank-1 A = g (x) rstd, B = g (x) u*rstd - b (x) 1
    built on the PE (grow = [1,D] gamma row, nbrow = [1,D] row of -beta)."""
    at = mybir.ActivationFunctionType
    with tc.tile_pool(name=tag, bufs=1) as pool:
        ps_u = pp.tile([1, SCH], F32, tag="ps")
        ps_v = pp.tile([1, SCH], F32, tag="ps")
        for dc in range(DC):
            nc.tensor.matmul(ps_u[:], invd[:], src[:, dc],
                             start=(dc == 0), stop=(dc == DC - 1))
        for dc in range(DC):
            nc.tensor.matmul(ps_v[:], invd[:], sq[:, dc],
                             start=(dc == 0), stop=(dc == DC - 1))
        u = pool.tile([1, SCH], DT)
        var = pool.tile([1, SCH], F32)
        sd = pool.tile([1, SCH], F32)
        rstd = pool.tile([1, SCH], DT)
        ur = pool.tile([1, SCH], DT)
        nc.vector.tensor_copy(u[:], ps_u[:])
        nc.vector.tensor_tensor(var[:], u[:], u[:], mybir.AluOpType.mult)
        nc.vector.tensor_tensor(var[:], ps_v[:], var[:], mybir.AluOpType.subtract)
        nc.scalar.activation(sd[:], var[:], at.Sqrt, bias=EPS)
        nc.vector.reciprocal(rstd[:], sd[:])
        nc.vector.tensor_tensor(ur[:], u[:], rstd[:], mybir.AluOpType.mult)
        for dc in range(DC):
            ps_a = ppacc.tile([P, SCH], F32, tag="acc")
            ps_b = pp2.tile([P, SCH], F32, tag="ps2")
            nc.tensor.matmul(ps_a[:], grow[:, ts(dc, P)], rstd[:],
                             start=True, stop=True)
            nc.tensor.matmul(ps_b[:], grow[:, ts(dc, P)], ur[:],
                             start=True, stop=False)
            nc.tensor.matmul(ps_b[:], nbrow[:, ts(dc, P)], onesw[0:1, 0:SCH],
                             start=False, stop=True)
            t = pool.tile([P, SCH], F32, tag="lnt", bufs=2)
            nc.vector.tensor_tensor(t[:], src[:, dc], ps_a[:],
                                    mybir.AluOpType.mult)
            nc.vector.tensor_tensor(dst[:, dc], t[:], ps_b[:],
                                    mybir.AluOpType.subtract)


def _build():
    at = mybir.ActivationFunctionType
    nc = bacc.Bacc("TRN2", target_bir_lowering=False)

    xT_d = nc.dram_tensor("xT", [P, DC, S], DT, kind="ExternalInput")
    xs_d = nc.dram_tensor("xs", [P, DC, SCH], DT, kind="ExternalInput")
    wq_d = nc.dram_tensor("wq", [P, DC, D], DT, kind="ExternalInput")
    wk_d = nc.dram_tensor("wk", [P, DC, D], DT, kind="ExternalInput")
    wv_d = nc.dram_tensor("wv", [P, DC, D], DT, kind="ExternalInput")
    wo_d = nc.dram_tensor("wo", [P, DC, D], DT, kind="ExternalInput")
    w1_d = nc.dram_tensor("w1", [P, DC, F], DT, kind="ExternalInput")
    w2_d = nc.dram_tensor("w2", [P, FC, D], DT, kind="ExternalInput")
    ones_d = nc.dram_tensor("ones_c", [P, 512], DT, kind="ExternalInput")
    bq_d = nc.dram_tensor("bq", [P, DC], F32, kind="ExternalInput")
    bk_d = nc.dram_tensor("bk", [P, DC], F32, kind="ExternalInput")
    bv_d = nc.dram_tensor("bvr", [1, D], DT, kind="ExternalInput")
    bo_d = nc.dram_tensor("bo", [P, DC], F32, kind="ExternalInput")
    bf1_d = nc.dram_tensor("bf1", [P, FC], F32, kind="ExternalInput")
    bf2_d = nc.dram_tensor("bf2", [P, DC], F32, kind="ExternalInput")
    g1_d = nc.dram_tensor("g1", [P, DC], F32, kind="ExternalInput")
    b1_d = nc.dram_tensor("b1", [P, DC], F32, kind="ExternalInput")
    g2_d = nc.dram_tensor("g2", [P, DC], F32, kind="ExternalInput")
    b2_d = nc.dram_tensor("b2", [P, DC], F32, kind="ExternalInput")
    gam_d = nc.dram_tensor("gam", [P, TC], F32, kind="ExternalInput")
    invd_d = nc.dram_tensor("invd", [P, 1], DT, kind="ExternalInput")
    g1r_d = nc.dram_tensor("g1r", [1, D], DT, kind="ExternalInput")
    nb1r_d = nc.dram_tensor("nb1r", [1, D], DT, kind="ExternalInput")
    g2r_d = nc.dram_tensor("g2r", [1, D], DT, kind="ExternalInput")
    nb2r_d = nc.dram_tensor("nb2r", [1, D], DT, kind="ExternalInput")
    gamh_d = nc.dram_tensor("gamh", [P, TC, H], DT, kind="ExternalInput")
    out_d = nc.dram_tensor("outT", [P, DC, SCH], F32, kind="ExternalOutput")

    import contextlib
    lp = (nc.allow_low_precision(reason="float32r operands are rounded by design")
          if USE_F32R else contextlib.nullcontext())
    with lp, tile.TileContext(nc) as tc:
        with tc.tile_pool(name="small", bufs=1) as small, \
             tc.tile_pool(name="ps", bufs=2, space="PSUM") as pp, \
             tc.tile_pool(name="ps2", bufs=2, space="PSUM") as pp2, \
             tc.tile_pool(name="psacc", bufs=2, space="PSUM") as ppacc:

            # ---- constants (only V-phase-critical ones issued up front) ----
            onesw = small.tile([P, 512], DT)
            bq_sb = small.tile([P, DC], F32)
            bk_sb = small.tile([P, DC], F32)
            bo_sb = small.tile([P, DC], F32)
            bf1_sb = small.tile([P, FC], F32)
            bf2_sb = small.tile([P, DC], F32)
            gam_sb = small.tile([P, TC], F32)
            invd = small.tile([P, 1], DT)
            bv_row = small.tile([1, D], DT)
            ones = onesw[:, 0:P]
            epsc = small.tile([P, 1], F32)
            nc.vector.memset(epsc[:], EPS)
            nc.const_aps.aps[(F32, EPS)] = epsc[:]

            # long-lived tiles, allocated in reverse order of death (LIFO pools)
            hT, hT_free = tc.tile([P, DC, SCH], DT, name="hT")

            # ================= Phase V =================
            # v stored [token, feature] with a ones column per head (for Z).
            vA, vA_free = tc.tile([P, TC, H * WA], DT, name="vA")
            vA_h = vA[:].rearrange("p t (h c) -> p t h c", c=WA)
            # gamma column per head (Z weights; = mask gamma, 1.0 for unmasked)
            gamh_sb = small.tile([P, TC, H], DT)
            nc.sync.dma_start(gamh_sb[:], gamh_d[:])
            nc.vector.tensor_copy(vA_h[:, :, :, W], gamh_sb[:])
            with tc.tile_pool(name="pv", bufs=1) as pv, \
                 tc.tile_pool(name="pvw", bufs=4) as pvw:
                wv_sb = pv.tile([P, DC, D], DT)
                # first-needed data first: halves of wv[0] + first token window
                nc.sync.dma_start(wv_sb[:, 0, 0:512], wv_d[:, 0, 0:512])
                xws = {0: pvw.tile([P, DC, P], DT, tag="xw", name="xw0")}
                nc.scalar.dma_start(xws[0][:, 0:2], xT_d[:, 0:2, ts(0, P)])
                nc.scalar.dma_start(xws[0][:, 2:], xT_d[:, 2:, ts(0, P)])
                nc.sync.dma_start(wv_sb[:, 0, 512:], wv_d[:, 0, 512:])
                nc.sync.dma_start(gam_sb[:], gam_d[:])
                nc.sync.dma_start(bv_row[:], bv_d[:])
                nc.sync.dma_start(onesw[:], ones_d[:])
                nc.sync.dma_start(invd[:], invd_d[:])
                for dc in range(1, DC):
                    nc.sync.dma_start(wv_sb[:, dc], wv_d[:, dc])
                for sb, dr in [(bq_sb, bq_d), (bk_sb, bk_d), (bo_sb, bo_d),
                               (bf1_sb, bf1_d), (bf2_sb, bf2_d)]:
                    nc.sync.dma_start(sb[:], dr[:])
                for tcl in range(TC):
                    if tcl in xws:
                        xw = xws[tcl]
                    else:
                        xw = pvw.tile([P, DC, P], DT, tag="xw", name="xw")
                        eng = nc.scalar if tcl % 2 == 0 else nc.sync
                        eng.dma_start(xw[:], xT_d[:, :, ts(tcl, P)])
                    for dvh in range(2):
                        psv = (ppacc.tile([P, 512], F32, tag="acc", name="psv")
                               if dvh == 0 else
                               pp.tile([P, 512], F32, tag="ps", name="psv2"))
                        for dc in range(DC):
                            nc.tensor.matmul(psv[:], xw[:, dc],
                                             wv_sb[:, dc, ts(dvh, 512)],
                                             start=(dc == 0), stop=False)
                        nc.tensor.matmul(psv[:], ones[0:1, 0:P],
                                         bv_row[:, ts(dvh, 512)],
                                         start=False, stop=True)
                        nc.vector.tensor_scalar(
                            vA_h[:, tcl, dvh * 8:(dvh + 1) * 8, 0:W],
                            psv[:].rearrange("p (h c) -> p h c", c=W),
                            gam_sb[:, tcl:tcl + 1], None, mybir.AluOpType.mult,
                        )

            # ================= Phase K =================
            # kT stored [feature, token].
            kT, kT_free = tc.tile([P, DC, S], DT, name="kT")
            with tc.tile_pool(name="pk", bufs=1) as pk, \
                 tc.tile_pool(name="pkw", bufs=2) as pkw:
                wk_sb = pk.tile([P, DC, D], DT)
                nc.sync.dma_start(wk_sb[:, 0, 0:P], wk_d[:, 0, 0:P])
                nc.scalar.dma_start(wk_sb[:, 0, P:], wk_d[:, 0, P:])
                for dc in range(1, DC):
                    nc.sync.dma_start(wk_sb[:, dc], wk_d[:, dc])
                for tw in range(S // 256):
                    if False:
                        xw = None
                    else:
                        xw = pkw.tile([P, DC, 256], DT, tag="xw", name="xwk")
                        eng = nc.scalar if tw % 2 == 0 else nc.sync
                        eng.dma_start(xw[:], xT_d[:, :, ts(tw, 256)])
                    for dk in range(DC):
                        psk = pp.tile([P, 256], F32, tag="ps")
                        for dc in range(DC):
                            nc.tensor.matmul(psk[:], wk_sb[:, dc, ts(dk, P)],
                                             xw[:, dc],
                                             start=(dc == 0), stop=(dc == DC - 1))
                        nc.vector.tensor_scalar(kT[:, dk, ts(tw, 256)], psk[:],
                                                bk_sb[:, dk:dk + 1], None,
                                                mybir.AluOpType.add)

            # ================= Phase Q =================
            qT, qT_free = tc.tile([P, DC, SCH], DT, name="qT")
            with tc.tile_pool(name="pq", bufs=1) as pq, \
                 tc.tile_pool(name="pqw", bufs=3) as pqw:
                xs = pq.tile([P, DC, SCH], DT)
                for dc in range(DC):
                    nc.scalar.dma_start(xs[:, dc], xs_d[:, dc])
                for dq in range(DC):
                    wt = pqw.tile([P, DC, P], DT, tag="wt")
                    nc.sync.dma_start(wt[:], wq_d[:, :, ts(dq, P)])
                    psq = pp.tile([P, SCH], F32, tag="ps")
                    for dc in range(DC):
                        nc.tensor.matmul(psq[:], wt[:, dc], xs[:, dc],
                                         start=(dc == 0), stop=(dc == DC - 1))
                    nc.vector.tensor_scalar(qT[:, dq], psq[:],
                                            bq_sb[:, dq:dq + 1], None,
                                            mybir.AluOpType.add)

            # ================= Attention =================
            with tc.tile_pool(name="pat", bufs=1) as pat:
                for h in range(H):
                    hc, hp = h // 2, W * (h % 2)
                    pso = ppacc.tile([WA, SCH], F32, tag="acc")
                    for tcp in range(TC // 2):
                        pss = pp2.tile([P, 2 * SCH], F32, tag="ps2")
                        for j in range(2):
                            nc.tensor.matmul(pss[:, ts(j, SCH)],
                                             kT[hp:hp + W, hc, ts(2 * tcp + j, P)],
                                             qT[hp:hp + W, hc],
                                             start=True, stop=True)
                        probs = pat.tile([P, 2 * SCH], DT, tag="probs", bufs=4)
                        nc.scalar.activation(probs[:], pss[:], at.Exp,
                                             scale=float(SCALE))
                        for j in range(2):
                            tcl = 2 * tcp + j
                            nc.tensor.matmul(pso[:],
                                             vA[:, tcl, h * WA:(h + 1) * WA],
                                             probs[:, ts(j, SCH)],
                                             start=(tcl == 0), stop=(tcl == TC - 1))
                    rz = pat.tile([P, SCH], DT, tag="rz", bufs=2)
                    nc.vector.reciprocal(rz[W:W + 1], pso[W:W + 1])
                    psb = pp.tile([W, SCH], F32, tag="ps")
                    nc.tensor.matmul(psb[:], ones[W:W + 1, 0:W], rz[W:W + 1],
                                     start=True, stop=True)
                    rzb = pat.tile([W, SCH], DT, tag="rzb", bufs=2)
                    nc.vector.tensor_copy(rzb[:], psb[:])
                    if hp == 0:
                        nc.vector.tensor_tensor(hT[0:W, hc], pso[0:W], rzb[:],
                                                mybir.AluOpType.mult)
                    else:
                        tn = pat.tile([W, SCH], DT, tag="ntmp", bufs=2)
                        nc.vector.tensor_tensor(tn[:], pso[0:W], rzb[:],
                                                mybir.AluOpType.mult)
                        nc.sync.dma_start(hT[hp:hp + W, hc], tn[:])
            qT_free()
            kT_free()
            vA_free()

            # ================= Out-proj + residual =================
            # FFN-side tiles are allocated first so their SBUF slots do not
            # overlap the out-proj/LN1 scratch - lets w1/w2 DMAs prefetch
            # while LN1 is still running.
            prow_cm = tc.tile_pool(name="prow", bufs=1)
            prow = prow_cm.__enter__()
            g1r = prow.tile([1, D], DT)
            nb1r = prow.tile([1, D], DT)
            nc.scalar.dma_start(g1r[:], g1r_d[:])
            nc.scalar.dma_start(nb1r[:], nb1r_d[:])
            h1T, h1T_free = tc.tile([P, DC, SCH], DT, name="h1T")
            r2T, r2T_free = tc.tile([P, DC, SCH], DT, name="r2T")
            g1T, g1T_free = tc.tile([P, FC, SCH], DT, name="g1T")
            pf1_cm = tc.tile_pool(name="pf1", bufs=2)
            pf1 = pf1_cm.__enter__()
            r1T, r1T_free = tc.tile([P, DC, SCH], DT, name="r1T")
            sq1, sq1_free = tc.tile([P, DC, SCH], DT, name="sq1")
            with tc.tile_pool(name="po", bufs=1) as po, \
                 tc.tile_pool(name="pow", bufs=2) as pow_:
                xs2 = po.tile([P, DC, SCH], DT)
                wts = [pow_.tile([P, DC, P], DT, tag="wt", name=f"wo{dp}")
                       for dp in range(2)]
                nc.sync.dma_start(wts[0][:], wo_d[:, :, ts(0, P)])
                nc.scalar.dma_start(wts[1][:], wo_d[:, :, ts(1, P)])
                for dc in range(DC):
                    nc.scalar.dma_start(xs2[:, dc], xs_d[:, dc])
                # warm the Sqrt table while ACT is otherwise idle
                sqwarm = po.tile([1, 1], F32)
                nc.scalar.activation(sqwarm[:], epsc[0:1, :], at.Sqrt)
                w1t0 = pf1.tile([P, DC, 2 * P], DT, tag="wt", name="w1t0")
                nc.sync.dma_start(w1t0[:], w1_d[:, :, ts(0, 2 * P)])
                for dp in range(DC):
                    if dp < 2:
                        wt = wts[dp]
                    else:
                        wt = pow_.tile([P, DC, P], DT, tag="wt", name="wo")
                        eng = nc.sync if dp % 2 == 0 else nc.scalar
                        eng.dma_start(wt[:], wo_d[:, :, ts(dp, P)])
                    psr = pp.tile([P, SCH], F32, tag="ps")
                    for dc in range(DC):
                        nc.tensor.matmul(psr[:], wt[:, dc], hT[:, dc],
                                         start=(dc == 0), stop=(dc == DC - 1))
                    nc.vector.tensor_scalar(r1T[:, dp], psr[:],
                                            bo_sb[:, dp:dp + 1], None,
                                            mybir.AluOpType.add)
                    nc.vector.tensor_tensor(r1T[:, dp], r1T[:, dp], xs2[:, dp],
                                            mybir.AluOpType.add)
                    nc.scalar.activation(sq1[:, dp], r1T[:, dp], at.Square)

            # ================= LN1 =================
            _layer_norm(nc, tc, pp, pp2, ppacc, onesw, invd, r1T, sq1, h1T, g1r, nb1r, "ln1")
            sq1_free()
            r1T_free()
            # reuse the row tiles for LN2's affine rows
            nc.scalar.dma_start(g1r[:], g2r_d[:])
            nc.scalar.dma_start(nb1r[:], nb2r_d[:])
            sq2, sq2_free = tc.tile([P, DC, SCH], DT, name="sq2")

            # ================= FFN =================
            with tc.tile_pool(name="pf2", bufs=3) as pf2:
                for fcp in range(FC // 2):
                    if fcp == 0:
                        w1t = w1t0
                    else:
                        w1t = pf1.tile([P, DC, 2 * P], DT, tag="wt")
                        nc.sync.dma_start(w1t[:], w1_d[:, :, ts(fcp, 2 * P)])
                    for j in range(2):
                        fc = 2 * fcp + j
                        psg = pp.tile([P, SCH], F32, tag="ps")
                        for dc in range(DC):
                            nc.tensor.matmul(psg[:], w1t[:, dc, ts(j, P)],
                                             h1T[:, dc],
                                             start=(dc == 0), stop=(dc == DC - 1))
                        nc.scalar.activation(g1T[:, fc], psg[:], at.Gelu,
                                             bias=bf1_sb[:, fc:fc + 1])
                sqwarm2 = pf2.tile([1, 1], F32)
                nc.scalar.activation(sqwarm2[:], epsc[0:1, :], at.Sqrt)
                for dp in range(DC):
                    w2t = pf2.tile([P, FC, P], DT, tag="wt")
                    nc.sync.dma_start(w2t[:, 0:FC // 2], w2_d[:, 0:FC // 2, ts(dp, P)])
                    nc.sync.dma_start(w2t[:, FC // 2:], w2_d[:, FC // 2:, ts(dp, P)])
                    psf = ppacc.tile([P, SCH], F32, tag="acc")
                    for fc in range(FC):
                        nc.tensor.matmul(psf[:], w2t[:, fc], g1T[:, fc],
                                         start=(fc == 0), stop=(fc == FC - 1))
                    nc.vector.tensor_scalar(r2T[:, dp], psf[:],
                                            bf2_sb[:, dp:dp + 1], None,
                                            mybir.AluOpType.add)
                    nc.vector.tensor_tensor(r2T[:, dp], r2T[:, dp], h1T[:, dp],
                                            mybir.AluOpType.add)
                    nc.scalar.activation(sq2[:, dp], r2T[:, dp], at.Square)
            # ================= LN2 + out =================
            oT, oT_free = tc.tile([P, DC, SCH], F32, name="oT")
            _layer_norm(nc, tc, pp, pp2, ppacc, onesw, invd, r2T, sq2, oT, g1r, nb1r, "ln2")
            for dc in range(DC):
                nc.scalar.dma_start(out_d[:, dc], oT[:, dc])
            oT_free()
            sq2_free()
            pf1_cm.__exit__(None, None, None)
            g1T_free()
            r2T_free()
            h1T_free()
            prow_cm.__exit__(None, None, None)
            hT_free()

    nc.compile()
    return nc


def kernel(**inputs):
    x = np.asarray(inputs["x"], dtype=np.float32)
    mask = np.asarray(inputs["mask"])
    f = {k: np.asarray(inputs[k], dtype=np.float32) for k in
         ["wq", "bq", "wk", "bk", "wv", "bv", "wo", "bo", "g1", "b1",
          "w1", "bf1", "w2", "bf2", "g2", "b2"]}

    if "nc" not in _cache:
        _cache["nc"] = _build()
    nc = _cache["nc"]

    def wlay(w, pc):  # [K, M] -> [P, K//P, M]
        return np.ascontiguousarray(w.reshape(pc, P, w.shape[1]).transpose(1, 0, 2))

    def blay(b):      # [M] -> [P, M//P]
        return np.ascontiguousarray(b.reshape(-1, P).T)

    shared = {
        "wq": wlay(f["wq"], DC), "wk": wlay(f["wk"], DC), "wv": wlay(f["wv"], DC),
        "wo": wlay(f["wo"], DC), "w1": wlay(f["w1"], DC), "w2": wlay(f["w2"], FC),
        "ones_c": np.ones((P, 512), np.float32),
        "invd": np.full((P, 1), 1.0 / D, np.float32),
        "g1r": f["g1"].reshape(1, D), "g2r": f["g2"].reshape(1, D),
        "nb1r": (-f["b1"]).reshape(1, D),
        "nb2r": (-f["b2"]).reshape(1, D),
        "bq": blay(f["bq"]), "bk": blay(f["bk"]), "bvr": f["bv"].reshape(1, D),
        "bo": blay(f["bo"]), "bf1": blay(f["bf1"]), "bf2": blay(f["bf2"]),
        "g1": blay(f["g1"]), "b1": blay(f["b1"]),
        "g2": blay(f["g2"]), "b2": blay(f["b2"]),
    }

    in_maps = []
    for c in range(8):
        b, sq = c // 4, c % 4
        xTb = np.ascontiguousarray(x[b].T.reshape(DC, P, S).transpose(1, 0, 2))
        mbias = (-10000.0 * (1.0 - mask[b].astype(np.float32))).reshape(TC, P).T
        m = dict(shared)
        m["xT"] = xTb
        m["xs"] = np.ascontiguousarray(xTb[:, :, sq * SCH:(sq + 1) * SCH])
        gam = np.exp(mbias).astype(np.float32)          # 1.0 unmasked, 0.0 masked
        m["gam"] = np.ascontiguousarray(gam)
        m["gamh"] = np.ascontiguousarray(
            np.broadcast_to(gam[:, :, None], (P, TC, H)))
        in_maps.append(m)

    res = run_bass_kernel_spmd(nc, in_maps, core_ids=list(range(8)))
    _cache["last_res"] = res

    out = np.empty((B, S, D), np.float32)
    for c in range(8):
        b, sq = c // 4, c % 4
        oT = res.results[c]["outT"]  # [P, DC, SCH]
        out[b, sq * SCH:(sq + 1) * SCH, :] = oT.transpose(2, 1, 0).reshape(SCH, D)
    return out



# revision 20
# speedup vs baseline: 1.3863x; 1.3863x over previous
"""Trainium2 Bass kernel for a dense transformer encoder layer.

Problem: B=2, S=2048, D=1024, H=16 heads (W=64), F=4096, fp32.

Sharding: 8 cores = 2 batches x 4 sequence chunks of 512 tokens. Each core
computes K/V for its batch's full sequence (replicated within its 4-core
batch group) and Q/attention/FFN for its own 512-token chunk. No collectives.

Dataflow: activations live TRANSPOSED in SBUF ([feature, token], feature on
partitions) so QKV projections, attention, output projection and both FFN
matmuls chain on the TensorEngine with no on-device transposes. The host
transposes x on the way in and the per-core 1024x512 output on the way out.

Precision: the projection and FFN matmuls run in fp8e4m3 DoubleRow mode
(2 contraction chunks per instruction, 0.5 PE cycles/row). Weights are
pre-scaled by 64 on the host so their ~0.02-sigma values land mid-range in
e4m3; the 64x (or 4096x) product scale is divided out during PSUM
evacuation. Attention scores/probs/PV stay float32r: q,k carry the 64x
weight scale into the scores matmul and exp() folds 1/4096 into its scale.
The attention output is renormalized by 64/Z so hT lands ~unit-scale in
fp8. Residuals, LayerNorm and softmax statistics stay fp32/f32r.

Softmax: score tiles are [key-token, query-token]. The additive -10000 mask
is folded multiplicatively into V and into the per-head Z column as
gamma_t = exp(-10000*(1-m_t)) (exactly 0/1 in fp32), so exp needs no bias
and pairs of key-chunks share one wide ACT call. The normalizer Z comes
free as a 65th gamma-column appended to each head of V (the attention-value
matmul emits it as PSUM row 64); normalization multiplies by a PE-broadcast
64/Z row. LayerNorm statistics ride 1/D-scaled ones-column matmuls; the
affine apply is two elementwise passes (split DVE/Pool) against PE-built
rank-1 tiles.

Bias algebra (exact): bk drops out of softmax (constant per query row);
bv commutes through attention into bo' = bo + bv @ wo (host-folded).
"""
import numpy as np
import concourse.bass as bass
from concourse import bacc
import concourse.mybir as mybir
import concourse.tile as tile
from concourse.bass import ts
from concourse.bass_utils import run_bass_kernel_spmd

P = 128
B, S, D, H, W, F = 2, 2048, 1024, 16, 64, 4096
DC = D // P            # 8 d-chunks
FC = F // P            # 32 f-chunks
TC = S // P            # 16 key-token chunks
SCH = 512              # tokens per core
EPS = 1e-12
SCALE = 1.0 / np.sqrt(np.float32(W))
WA = W + 1             # per-head V columns incl. ones column
WS = 64.0              # host-side fp8 weight pre-scale

F32 = mybir.dt.float32
DT = mybir.dt.float32r
F8 = mybir.dt.float8e4
F8L = mybir.dt.float8e5
DRM = mybir.MatmulPerfMode.DoubleRow

_cache = {}


def _layer_norm(nc, tc, pp, pp2, ppacc, invd, urow2, src, sq, dst,
                grow, gbrow, tag, dst8=None, dst8b=None):
    """src/sq/dst: [P, DC, SCH] sbuf (feature on partitions). LN over features.
    sq = src*src comes from the caller's producing evacuation. Mean scaling
    rides the stats matmuls via the invd column. The apply is two elementwise
    passes: dst = src*A - B with rank-1 A = g (x) rstd and rank-2
    B = g (x) u*rstd - b (x) 1, the latter built in ONE K=2 matmul from
    gbrow = [g; -b] rows and urow2 = [u*rstd; ones] rows. Chunks alternate
    DVE/Pool so neither engine serializes the apply. dst8, if given, gets an
    fp8 copy of dst (for the following fp8 matmul) on the opposite engine."""
    at = mybir.ActivationFunctionType
    with tc.tile_pool(name=tag, bufs=1) as pool:
        ps_u = pp.tile([1, SCH], F32, tag="ps")
        ps_v = pp.tile([1, SCH], F32, tag="ps")
        for dc in range(DC):
            nc.tensor.matmul(ps_u[:], invd[:], src[:, dc],
                             start=(dc == 0), stop=(dc == DC - 1))
        for dc in range(DC):
            nc.tensor.matmul(ps_v[:], invd[:], sq[:, dc],
                             start=(dc == 0), stop=(dc == DC - 1))
        u = pool.tile([1, SCH], DT)
        var = pool.tile([1, SCH], F32)
        sd = pool.tile([1, SCH], F32)
        rstd = pool.tile([1, SCH], DT)
        nc.vector.tensor_copy(u[:], ps_u[:])
        nc.vector.tensor_tensor(var[:], u[:], u[:], mybir.AluOpType.mult)
        nc.vector.tensor_tensor(var[:], ps_v[:], var[:], mybir.AluOpType.subtract)
        nc.scalar.activation(sd[:], var[:], at.Sqrt, bias=EPS)
        nc.vector.reciprocal(rstd[:], sd[:])
        nc.vector.tensor_tensor(urow2[0:1], u[:], rstd[:], mybir.AluOpType.mult)
        for dc in range(DC):
            ps_a = ppacc.tile([P, SCH], F32, tag="acc")
            ps_b = pp2.tile([P, SCH], F32, tag="ps2")
            nc.tensor.matmul(ps_a[:], grow[:, ts(dc, P)], rstd[:],
                             start=True, stop=True)
            nc.tensor.matmul(ps_b[:], gbrow[0:2, ts(dc, P)], urow2[0:2],
                             start=True, stop=True)
            t = pool.tile([P, SCH], F32, tag="lnt", bufs=4)
            nc.vector.tensor_tensor(t[:], src[:, dc], ps_a[:],
                                    mybir.AluOpType.mult)
            nc.vector.tensor_tensor(dst[:, dc], t[:], ps_b[:],
                                    mybir.AluOpType.subtract)
            if dst8 is not None:
                nc.gpsimd.tensor_copy(dst8[:, dc], dst[:, dc])
                nc.vector.tensor_tensor(dst8b[:, dc], dst[:, dc], dst8[:, dc],
                                        mybir.AluOpType.subtract)


def _build():
    at = mybir.ActivationFunctionType
    nc = bacc.Bacc("TRN2", target_bir_lowering=False)

    xT8_d = nc.dram_tensor("xT8", [P, DC, S], F8, kind="ExternalInput")
    xs8_d = nc.dram_tensor("xs8", [P, DC, SCH], F8, kind="ExternalInput")
    xs_d = nc.dram_tensor("xs", [P, DC, SCH], DT, kind="ExternalInput")
    wq_d = nc.dram_tensor("wq8", [P, DC, D], F8, kind="ExternalInput")
    wk_d = nc.dram_tensor("wk8", [P, DC, D], F8, kind="ExternalInput")
    wv_d = nc.dram_tensor("wv8", [P, DC, D], F8, kind="ExternalInput")
    wo_d = nc.dram_tensor("wo8", [P, DC, D], F8, kind="ExternalInput")
    w1_d = nc.dram_tensor("w18", [P, DC, F], F8, kind="ExternalInput")
    w1l_d = nc.dram_tensor("w18l", [P, DC, F], F8L, kind="ExternalInput")
    w2_d = nc.dram_tensor("w28", [P, FC, D], F8, kind="ExternalInput")
    w2l_d = nc.dram_tensor("w28l", [P, FC, D], F8L, kind="ExternalInput")
    bq_d = nc.dram_tensor("bq64", [P, DC], F32, kind="ExternalInput")
    bo_d = nc.dram_tensor("boP", [P, DC], F32, kind="ExternalInput")
    bf1_d = nc.dram_tensor("bf1", [P, FC], F32, kind="ExternalInput")
    bf2_d = nc.dram_tensor("bf2", [P, DC], F32, kind="ExternalInput")
    gam_d = nc.dram_tensor("gam64", [P, TC], F32, kind="ExternalInput")
    invd_d = nc.dram_tensor("invd", [P, 1], DT, kind="ExternalInput")
    g1r_d = nc.dram_tensor("g1r", [1, D], DT, kind="ExternalInput")
    g2r_d = nc.dram_tensor("g2r", [1, D], DT, kind="ExternalInput")
    gb1_d = nc.dram_tensor("gb1", [2, D], DT, kind="ExternalInput")
    gb2_d = nc.dram_tensor("gb2", [2, D], DT, kind="ExternalInput")
    gamh_d = nc.dram_tensor("gamh", [P, TC, H], DT, kind="ExternalInput")
    ones_d = nc.dram_tensor("ones_c", [P, 512], DT, kind="ExternalInput")
    out_d = nc.dram_tensor("outT", [P, DC, SCH], F32, kind="ExternalOutput")

    with nc.allow_low_precision(reason="fp8/f32r matmuls are rounded by design"), \
         tile.TileContext(nc) as tc:
        with tc.tile_pool(name="small", bufs=1) as small, \
             tc.tile_pool(name="ps", bufs=2, space="PSUM") as pp, \
             tc.tile_pool(name="ps2", bufs=2, space="PSUM") as pp2, \
             tc.tile_pool(name="psacc", bufs=2, space="PSUM") as ppacc:

            # ---- small constants ----
            bq_sb = small.tile([P, DC], F32)
            bo_sb = small.tile([P, DC], F32)
            bf1_sb = small.tile([P, FC], F32)
            bf2_sb = small.tile([P, DC], F32)
            gam_sb = small.tile([P, TC], F32)
            invd = small.tile([P, 1], DT)
            onesw = small.tile([P, 512], DT)
            urow2 = small.tile([2, SCH], DT)
            epsc = small.tile([P, 1], F32)
            gelw = small.tile([1, 1], F32)
            # urow2 row 1 stays ones; row 0 is rewritten by each LN
            nc.sync.dma_start(onesw[:], ones_d[:])
            nc.sync.dma_start(urow2[0:2], ones_d[0:2, 0:SCH])
            nc.vector.memset(epsc[:], EPS)
            nc.const_aps.aps[(F32, EPS)] = epsc[:]

            # long-lived tiles, allocated in reverse order of death (LIFO pools)
            hT, hT_free = tc.tile([P, DC, SCH], F8, name="hT")
            vA, vA_free = tc.tile([P, TC, H * WA], DT, name="vA")
            vA_h = vA[:].rearrange("p t (h c) -> p t h c", c=WA)
            kT, kT_free = tc.tile([P, DC, S], DT, name="kT")
            qT, qT_free = tc.tile([P, DC, SCH], DT, name="qT")
            xT8, xT8_free = tc.tile([P, DC, S], F8, name="xT8")

            # gamma column per head (Z weights; = mask gamma, 1.0 for unmasked)
            gamh_sb = small.tile([P, TC, H], DT)
            nc.sync.dma_start(gamh_sb[:], gamh_d[:])
            nc.vector.tensor_copy(vA_h[:, :, :, W], gamh_sb[:])

            # ================= Phase V =================
            # v stored [token, feature] with a ones column per head (for Z).
            with tc.tile_pool(name="pv", bufs=1) as pv:
                wv8 = pv.tile([P, DC, D], F8)
                wk8 = pv.tile([P, DC, D], F8)
                wq8 = pv.tile([P, DC, D], F8)
                xs8 = pv.tile([P, DC, SCH], F8)
                # first-needed data first: x chunks 0-1 + wv first half
                nc.sync.dma_start(xT8[:, 0:2], xT8_d[:, 0:2])
                nc.scalar.dma_start(wv8[:, 0:2], wv_d[:, 0:2])
                nc.sync.dma_start(xT8[:, 2:4], xT8_d[:, 2:4])
                nc.scalar.dma_start(wv8[:, 2:4], wv_d[:, 2:4])
                nc.sync.dma_start(xT8[:, 4:8], xT8_d[:, 4:8])
                nc.scalar.dma_start(wv8[:, 4:8], wv_d[:, 4:8])
                nc.sync.dma_start(gam_sb[:], gam_d[:])
                nc.sync.dma_start(invd[:], invd_d[:])
                nc.scalar.dma_start(wk8[:], wk_d[:])
                nc.sync.dma_start(wq8[:], wq_d[:])
                nc.scalar.dma_start(xs8[:], xs8_d[:])
                for sb, dr in [(bq_sb, bq_d), (bo_sb, bo_d),
                               (bf1_sb, bf1_d), (bf2_sb, bf2_d)]:
                    nc.sync.dma_start(sb[:], dr[:])
                for tcl in range(TC):
                    for dvh in range(2):
                        psv = (ppacc.tile([P, 512], F32, tag="acc", name="psv")
                               if dvh == 0 else
                               pp.tile([P, 512], F32, tag="ps", name="psv2"))
                        for i in range(DC // 2):
                            nc.tensor.matmul(psv[:],
                                             xT8[:, 2 * i:2 * i + 2, ts(tcl, P)],
                                             wv8[:, 2 * i:2 * i + 2, ts(dvh, 512)],
                                             start=(i == 0), stop=(i == DC // 2 - 1),
                                             perf_mode=DRM)
                        # gpsimd cannot touch PSUM: split evacuations DVE/ACT
                        if dvh == 0:
                            nc.vector.tensor_scalar(
                                vA_h[:, tcl, 0:8, 0:W],
                                psv[:].rearrange("p (h c) -> p h c", c=W),
                                gam_sb[:, tcl:tcl + 1], None,
                                mybir.AluOpType.mult,
                            )
                        else:
                            nc.scalar.activation(
                                vA_h[:, tcl, 8:16, 0:W],
                                psv[:].rearrange("p (h c) -> p h c", c=W),
                                at.Copy, scale=gam_sb[:, tcl:tcl + 1],
                            )

                # ================= Phase K =================
                # kT stored [feature, token], carrying the 64x weight scale.
                for tw in range(S // 512):
                    for dk in range(DC):
                        psk = (pp.tile([P, 512], F32, tag="ps", name="psk")
                               if dk % 2 == 0 else
                               ppacc.tile([P, 512], F32, tag="acc", name="psk2"))
                        for i in range(DC // 2):
                            nc.tensor.matmul(psk[:],
                                             wk8[:, 2 * i:2 * i + 2, ts(dk, P)],
                                             xT8[:, 2 * i:2 * i + 2, ts(tw, 512)],
                                             start=(i == 0), stop=(i == DC // 2 - 1),
                                             perf_mode=DRM)
                        if dk % 2 == 0:
                            nc.vector.tensor_copy(kT[:, dk, ts(tw, 512)], psk[:])
                        else:
                            nc.scalar.activation(kT[:, dk, ts(tw, 512)], psk[:],
                                                 at.Copy)

                # ================= Phase Q =================
                # qT carries the 64x weight scale (bq pre-scaled to match).
                for dq in range(DC):
                    psq = (pp.tile([P, SCH], F32, tag="ps", name="psq")
                           if dq % 2 == 0 else
                           ppacc.tile([P, SCH], F32, tag="acc", name="psq2"))
                    for i in range(DC // 2):
                        nc.tensor.matmul(psq[:],
                                         wq8[:, 2 * i:2 * i + 2, ts(dq, P)],
                                         xs8[:, 2 * i:2 * i + 2],
                                         start=(i == 0), stop=(i == DC // 2 - 1),
                                         perf_mode=DRM)
                    if dq % 2 == 0:
                        nc.vector.tensor_scalar(qT[:, dq], psq[:],
                                                bq_sb[:, dq:dq + 1], None,
                                                mybir.AluOpType.add)
                    else:
                        nc.scalar.activation(qT[:, dq], psq[:], at.Identity,
                                             bias=bq_sb[:, dq:dq + 1])
            xT8_free()

            # ================= Attention =================
            # scores psum = (64q)·(64k) = 4096*s; exp folds SCALE/4096.
            # hT = pso * (64/Z) lands unit-scale in fp8.
            with tc.tile_pool(name="pat", bufs=1) as pat:
                for h in range(H):
                    hc, hp = h // 2, W * (h % 2)
                    pso = ppacc.tile([WA, SCH], F32, tag="acc")
                    for tcp in range(TC // 2):
                        pss = pp2.tile([P, 2 * SCH], F32, tag="ps2")
                        for j in range(2):
                            nc.tensor.matmul(pss[:, ts(j, SCH)],
                                             kT[hp:hp + W, hc, ts(2 * tcp + j, P)],
                                             qT[hp:hp + W, hc],
                                             start=True, stop=True)
                        probs = pat.tile([P, 2 * SCH], DT, tag="probs", bufs=4)
                        nc.scalar.activation(probs[:], pss[:], at.Exp,
                                             scale=float(SCALE / (WS * WS)))
                        for j in range(2):
                            tcl = 2 * tcp + j
                            nc.tensor.matmul(pso[:],
                                             vA[:, tcl, h * WA:(h + 1) * WA],
                                             probs[:, ts(j, SCH)],
                                             start=(tcl == 0), stop=(tcl == TC - 1))
                    rz = pat.tile([P, SCH], DT, tag="rz", bufs=2)
                    nc.vector.reciprocal(rz[W:W + 1], pso[W:W + 1])
                    psb = pp.tile([W, SCH], F32, tag="ps")
                    nc.tensor.matmul(psb[:], onesw[W:W + 1, 0:W], rz[W:W + 1],
                                     start=True, stop=True)
                    rzb = pat.tile([W, SCH], DT, tag="rzb", bufs=2)
                    nc.vector.tensor_copy(rzb[:], psb[:])
                    if hp == 0:
                        nc.vector.tensor_tensor(hT[0:W, hc], pso[0:W], rzb[:],
                                                mybir.AluOpType.mult)
                    else:
                        tn = pat.tile([W, SCH], F8, tag="ntmp", bufs=2)
                        nc.vector.tensor_tensor(tn[:], pso[0:W], rzb[:],
                                                mybir.AluOpType.mult)
                        nc.sync.dma_start(hT[hp:hp + W, hc], tn[:])
            qT_free()
            kT_free()
            vA_free()

            # ================= Out-proj + residual =================
            # FFN-side tiles are allocated first so their SBUF slots do not
            # overlap the out-proj/LN1 scratch - lets w1 DMA prefetch while
            # LN1 is still running.
            prow_cm = tc.tile_pool(name="prow", bufs=1)
            prow = prow_cm.__enter__()
            g1r = prow.tile([1, D], DT)
            gb1 = prow.tile([2, D], DT)
            nc.scalar.dma_start(g1r[:], g1r_d[:])
            nc.scalar.dma_start(gb1[:], gb1_d[:])
            # w1 hi/lo stream in 4-column-chunk tiles (w1 resident would cost
            # 64KB/partition); first group prefetched during out-proj
            pf1_cm = tc.tile_pool(name="pf1", bufs=3)
            pf1 = pf1_cm.__enter__()
            w1t0 = pf1.tile([P, DC, 4 * P], F8, tag="wt", name="w1t0")
            w1lt0 = pf1.tile([P, DC, 4 * P], F8L, tag="wtl", name="w1lt0")
            h1f, h1f_free = tc.tile([P, DC, SCH], DT, name="h1f")
            h18, h18_free = tc.tile([P, DC, SCH], F8, name="h18")
            h18b, h18b_free = tc.tile([P, DC, SCH], F8L, name="h18b")
            g8b, g8b_free = tc.tile([P, FC, SCH], F8L, name="g8b")
            r2T, r2T_free = tc.tile([P, DC, SCH], DT, name="r2T")
            g1T8, g1T8_free = tc.tile([P, FC, SCH], F8, name="g1T8")
            r1T, r1T_free = tc.tile([P, DC, SCH], DT, name="r1T")
            sq1, sq1_free = tc.tile([P, DC, SCH], DT, name="sq1")
            with tc.tile_pool(name="po", bufs=1) as po, \
                 tc.tile_pool(name="pow", bufs=1) as pow_:
                xs2 = po.tile([P, DC, SCH], DT)
                wo8 = pow_.tile([P, DC, D], F8)
                nc.sync.dma_start(wo8[:, 0:4], wo_d[:, 0:4])
                nc.scalar.dma_start(wo8[:, 4:8], wo_d[:, 4:8])
                for dc in range(DC):
                    nc.scalar.dma_start(xs2[:, dc], xs_d[:, dc])
                # warm the Sqrt table while ACT is otherwise idle
                sqwarm = po.tile([1, 1], F32)
                nc.scalar.activation(sqwarm[:], epsc[0:1, :], at.Sqrt)
                nc.sync.dma_start(w1t0[:], w1_d[:, :, ts(0, 4 * P)])
                nc.scalar.dma_start(w1lt0[:], w1l_d[:, :, ts(0, 4 * P)])
                for dp in range(DC):
                    psr = (pp.tile([P, SCH], F32, tag="ps", name="psr")
                           if dp % 2 == 0 else
                           ppacc.tile([P, SCH], F32, tag="acc", name="psr2"))
                    for i in range(DC // 2):
                        nc.tensor.matmul(psr[:],
                                         wo8[:, 2 * i:2 * i + 2, ts(dp, P)],
                                         hT[:, 2 * i:2 * i + 2],
                                         start=(i == 0), stop=(i == DC // 2 - 1),
                                         perf_mode=DRM)
                    if dp % 2 == 0:
                        nc.vector.tensor_scalar(r1T[:, dp], psr[:],
                                                float(1.0 / (WS * WS)),
                                                bo_sb[:, dp:dp + 1],
                                                mybir.AluOpType.mult,
                                                mybir.AluOpType.add)
                    else:
                        nc.scalar.activation(r1T[:, dp], psr[:], at.Identity,
                                             scale=float(1.0 / (WS * WS)),
                                             bias=bo_sb[:, dp:dp + 1])
                    nc.gpsimd.tensor_tensor(r1T[:, dp], r1T[:, dp], xs2[:, dp],
                                            mybir.AluOpType.add)
                    nc.gpsimd.tensor_tensor(sq1[:, dp], r1T[:, dp], r1T[:, dp],
                                            mybir.AluOpType.mult)

            # ================= LN1 =================
            _layer_norm(nc, tc, pp, pp2, ppacc, invd, urow2, r1T, sq1, h1f,
                        g1r, gb1, "ln1", dst8=h18, dst8b=h18b)
            sq1_free()
            r1T_free()
            # reuse the row tiles for LN2's affine rows; warm the Gelu table
            # while the LN1 apply is still draining on DVE/Pool
            nc.scalar.activation(gelw[:], epsc[0:1, :], at.Gelu)
            nc.scalar.dma_start(g1r[:], g2r_d[:])
            nc.scalar.dma_start(gb1[:], gb2_d[:])
            sq2, sq2_free = tc.tile([P, DC, SCH], DT, name="sq2")

            # ================= FFN =================
            with tc.tile_pool(name="pf2", bufs=3) as pf2, \
                 tc.tile_pool(name="pgs", bufs=4) as pgs:
                for fcg in range(FC // 4):
                    if fcg == 0:
                        w1t, w1lt = w1t0, w1lt0
                    else:
                        w1t = pf1.tile([P, DC, 4 * P], F8, tag="wt")
                        w1lt = pf1.tile([P, DC, 4 * P], F8L, tag="wtl")
                        nc.sync.dma_start(w1t[:], w1_d[:, :, ts(fcg, 4 * P)])
                        nc.scalar.dma_start(w1lt[:], w1l_d[:, :, ts(fcg, 4 * P)])
                    for j in range(4):
                        fc = 4 * fcg + j
                        psg = (pp.tile([P, SCH], F32, tag="ps", name="psg")
                               if fc % 2 == 0 else
                               ppacc.tile([P, SCH], F32, tag="acc", name="psg2"))
                        for i in range(DC // 2):
                            nc.tensor.matmul(psg[:],
                                             w1t[:, 2 * i:2 * i + 2, ts(j, P)],
                                             h18[:, 2 * i:2 * i + 2],
                                             start=(i == 0), stop=False,
                                             perf_mode=DRM)
                        for i in range(DC // 2):
                            nc.tensor.matmul(psg[:],
                                             w1t[:, 2 * i:2 * i + 2, ts(j, P)],
                                             h18b[:, 2 * i:2 * i + 2],
                                             start=False, stop=False,
                                             perf_mode=DRM)
                        for i in range(DC // 2):
                            nc.tensor.matmul(psg[:],
                                             w1lt[:, 2 * i:2 * i + 2, ts(j, P)],
                                             h18[:, 2 * i:2 * i + 2],
                                             start=False, stop=(i == DC // 2 - 1),
                                             perf_mode=DRM)
                        gf = pgs.tile([P, SCH], DT, tag="gf")
                        nc.scalar.activation(gf[:], psg[:], at.Gelu,
                                             bias=bf1_sb[:, fc:fc + 1],
                                             scale=float(1.0 / WS))
                        nc.gpsimd.tensor_copy(g1T8[:, fc], gf[:])
                        nc.vector.tensor_tensor(g8b[:, fc], gf[:],
                                                g1T8[:, fc],
                                                mybir.AluOpType.subtract)
                sqwarm2 = pf2.tile([1, 1], F32)
                nc.scalar.activation(sqwarm2[:], epsc[0:1, :], at.Sqrt)
                for dp in range(DC):
                    w2t = pf2.tile([P, FC, P], F8, tag="wt")
                    w2tl = pf2.tile([P, FC, P], F8L, tag="wtl")
                    nc.sync.dma_start(w2t[:, 0:FC // 2], w2_d[:, 0:FC // 2, ts(dp, P)])
                    nc.sync.dma_start(w2t[:, FC // 2:], w2_d[:, FC // 2:, ts(dp, P)])
                    nc.sync.dma_start(w2tl[:, 0:FC // 2], w2l_d[:, 0:FC // 2, ts(dp, P)])
                    nc.sync.dma_start(w2tl[:, FC // 2:], w2l_d[:, FC // 2:, ts(dp, P)])
                    psf = (pp2.tile([P, SCH], F32, tag="ps2", name="psf")
                           if dp % 2 == 0 else
                           ppacc.tile([P, SCH], F32, tag="acc", name="psf2"))
                    for i in range(FC // 2):
                        nc.tensor.matmul(psf[:],
                                         w2t[:, 2 * i:2 * i + 2],
                                         g1T8[:, 2 * i:2 * i + 2],
                                         start=(i == 0), stop=False,
                                         perf_mode=DRM)
                    for i in range(FC // 2):
                        nc.tensor.matmul(psf[:],
                                         w2t[:, 2 * i:2 * i + 2],
                                         g8b[:, 2 * i:2 * i + 2],
                                         start=False, stop=False,
                                         perf_mode=DRM)
                    for i in range(FC // 2):
                        nc.tensor.matmul(psf[:],
                                         w2tl[:, 2 * i:2 * i + 2],
                                         g1T8[:, 2 * i:2 * i + 2],
                                         start=False, stop=(i == FC // 2 - 1),
                                         perf_mode=DRM)
                    if dp % 2 == 0:
                        nc.vector.tensor_scalar(r2T[:, dp], psf[:],
                                                float(1.0 / WS),
                                                bf2_sb[:, dp:dp + 1],
                                                mybir.AluOpType.mult,
                                                mybir.AluOpType.add)
                    else:
                        nc.scalar.activation(r2T[:, dp], psf[:], at.Identity,
                                             scale=float(1.0 / WS),
                                             bias=bf2_sb[:, dp:dp + 1])
                    nc.gpsimd.tensor_tensor(r2T[:, dp], r2T[:, dp], h1f[:, dp],
                                            mybir.AluOpType.add)
                    nc.gpsimd.tensor_tensor(sq2[:, dp], r2T[:, dp], r2T[:, dp],
                                            mybir.AluOpType.mult)
            # ================= LN2 + out =================
            oT, oT_free = tc.tile([P, DC, SCH], F32, name="oT")
            _layer_norm(nc, tc, pp, pp2, ppacc, invd, urow2, r2T, sq2, oT,
                        g1r, gb1, "ln2")
            for dc in range(DC):
                nc.scalar.dma_start(out_d[:, dc], oT[:, dc])
            oT_free()
            sq2_free()
            g1T8_free()
            r2T_free()
            g8b_free()
            h18b_free()
            h18_free()
            h1f_free()
            pf1_cm.__exit__(None, None, None)
            prow_cm.__exit__(None, None, None)
            hT_free()

    nc.compile()
    return nc


def kernel(**inputs):
    x = np.asarray(inputs["x"], dtype=np.float32)
    mask = np.asarray(inputs["mask"])
    f = {k: np.asarray(inputs[k], dtype=np.float32) for k in
         ["wq", "bq", "wk", "bk", "wv", "bv", "wo", "bo", "g1", "b1",
          "w1", "bf1", "w2", "bf2", "g2", "b2"]}

    if "nc" not in _cache:
        _cache["nc"] = _build()
    nc = _cache["nc"]

    f8 = mybir.dt.np(F8)
    f8l = mybir.dt.np(F8L)

    def wlay8(w, pc):  # [K, M] -> [P, K//P, M], fp8 with 64x pre-scale
        a = (w * WS).astype(f8)
        return np.ascontiguousarray(a.reshape(pc, P, w.shape[1]).transpose(1, 0, 2))

    def wlay8lo(w, pc):  # e5m2 residual of the e4m3 hi part (same 64x scale)
        hi = (w * WS).astype(f8).astype(np.float32)
        a = (w * WS - hi).astype(f8l)
        return np.ascontiguousarray(a.reshape(pc, P, w.shape[1]).transpose(1, 0, 2))

    def blay(b):      # [M] -> [P, M//P]
        return np.ascontiguousarray(b.reshape(-1, P).T)

    shared = {
        "wq8": wlay8(f["wq"], DC), "wk8": wlay8(f["wk"], DC),
        "wv8": wlay8(f["wv"], DC), "wo8": wlay8(f["wo"], DC),
        "w18": wlay8(f["w1"], DC), "w28": wlay8(f["w2"], FC),
        "w18l": wlay8lo(f["w1"], DC), "w28l": wlay8lo(f["w2"], FC),
        "invd": np.full((P, 1), 1.0 / D, np.float32),
        "g1r": f["g1"].reshape(1, D), "g2r": f["g2"].reshape(1, D),
        "gb1": np.stack([f["g1"], -f["b1"]]),
        "gb2": np.stack([f["g2"], -f["b2"]]),
        "bq64": blay(f["bq"]) * np.float32(WS),
        "boP": blay(f["bo"] + f["bv"] @ f["wo"]),
        "bf1": blay(f["bf1"]), "bf2": blay(f["bf2"]),
        "ones_c": np.ones((P, 512), np.float32),
    }

    in_maps = []
    for c in range(8):
        b, sq = c // 4, c % 4
        xTb = np.ascontiguousarray(x[b].T.reshape(DC, P, S).transpose(1, 0, 2))
        xT8 = xTb.astype(f8)
        mbias = (-10000.0 * (1.0 - mask[b].astype(np.float32))).reshape(TC, P).T
        m = dict(shared)
        m["xT8"] = xT8
        m["xs8"] = np.ascontiguousarray(xT8[:, :, sq * SCH:(sq + 1) * SCH])
        m["xs"] = np.ascontiguousarray(xTb[:, :, sq * SCH:(sq + 1) * SCH])
        gam = np.exp(mbias).astype(np.float32)          # 1.0 unmasked, 0.0 masked
        m["gam64"] = np.ascontiguousarray(gam / np.float32(WS))
        m["gamh"] = np.ascontiguousarray(
            np.broadcast_to(gam[:, :, None] / np.float32(WS), (P, TC, H)))
        in_maps.append(m)

    res = run_bass_kernel_spmd(nc, in_maps, core_ids=list(range(8)))
    _cache["last_res"] = res

    out = np.empty((B, S, D), np.float32)
    for c in range(8):
        b, sq = c // 4, c % 4
        oT = res.results[c]["outT"]  # [P, DC, SCH]
        out[b, sq * SCH:(sq + 1) * SCH, :] = oT.transpose(2, 1, 0).reshape(SCH, D)
    return out
